# revision 24
# baseline (speedup 1.0000x reference)
"""Trainium2 Bass kernel for the YOLO-style DetectionLayer.

Reference computation (per batch b, anchor a, grid cell (gy, gx)):
    pred = x[b].reshape(3, 85, 76, 76)  channels-first per anchor
    bx = (sigmoid(tx) + gx) * stride        stride = 608/76 = 8
    by = (sigmoid(ty) + gy) * stride
    bw = exp(tw) * anchor_w                 (stride cancels)
    bh = exp(th) * anchor_h
    conf/cls = sigmoid(...)
    out[b, a*5776 + gy*76 + gx, :] = [bx, by, bw, bh, conf, cls0..79]

Strategy (pure data-parallel over batch, 8 cores x 4 images):
  * Per (b, a) slab: DMA [85 ch, 5776 px] -> SBUF (channels on partitions).
  * One ACT pass: sigmoid over all 85 rows (single table set for the whole
    kernel -- exp is derived on DVE as s/(1-s) to avoid the ~2.7us ACT
    table switch between the sigmoid and exp sets).
  * TensorE transpose-mode matmuls flip [85, 128px] -> PSUM [128px, 85ch].
    Pixels are interleaved stride-6 so each SBUF output partition holds 6
    consecutive output rows = 2040 contiguous bytes in DRAM per partition
    (ideal DMA burst size).
  * Box fix-ups run in the transposed layout where box channels are a few
    free-dim columns across all 128 partitions (3-4 DVE ops per slab).
  * One big store DMA per slab, fully contiguous destination.
"""

from contextlib import ExitStack

import numpy as np

import concourse.bacc as bacc
import concourse.mybir as mybir
import concourse.tile as tile
from concourse.bass_utils import run_bass_kernel_spmd

F32 = mybir.dt.float32
Alu = mybir.AluOpType
Act = mybir.ActivationFunctionType

N_CORES = 8
NA = 3  # anchors
NCH = 85  # 5 + 80 classes
G = 76
GG = G * G  # 5776
STRIDE = 8.0

# pixel chunking for the transpose: 7 chunks of 128 partitions x 6 px
# (stride-6 interleave), tail chunk of 100 partitions x 4 px.
NJ, KI, KK = 7, 128, 6  # main: 7 * 768 px
TI, TK = 100, 4  # tail: 400 px
MAIN_PX = NJ * KI * KK  # 5376
MAIN_COLS = KK * NCH  # 510
TAIL_COLS = TK * NCH  # 340
OUT_COLS = NJ * MAIN_COLS + TAIL_COLS  # 3910

# grid8 / inva column layout: main j<7: q = j*12 + kk*2 + c ; tail: 84 + kk*2 + c
QCOLS = NJ * KK * 2 + TK * 2  # 92


def _build(
    nb: int,
    inp_bufs: int = 2,
    sig_bufs: int = 2,
    out_bufs: int = 3,
    ps_bufs: int = 4,
    copy_split: bool = False,
    sig_chunks: int = 2,
    in_engine: str = "gpsimd",
    wide_in: bool = False,
    base_alt: bool = False,
):
    nc = bacc.Bacc(
        "TRN2", target_bir_lowering=False, debug=False, enable_asserts=False
    )
    x = nc.dram_tensor("x", [nb, NA * NCH, GG], F32, kind="ExternalInput")
    g8 = nc.dram_tensor("grid8", [128, QCOLS], F32, kind="ExternalInput")
    iva = nc.dram_tensor("inva", [128, NA * QCOLS], F32, kind="ExternalInput")
    idn = nc.dram_tensor("ident", [NCH, NCH], F32, kind="ExternalInput")
    out = nc.dram_tensor("out", [nb, NA, GG, NCH], F32, kind="ExternalOutput")

    with tile.TileContext(nc) as tc, ExitStack() as ctx:
        cpool = ctx.enter_context(tc.tile_pool(name="consts", bufs=1))
        inp = ctx.enter_context(tc.tile_pool(name="inp", bufs=inp_bufs))
        sp = ctx.enter_context(tc.tile_pool(name="sig", bufs=sig_bufs))
        op = ctx.enter_context(tc.tile_pool(name="outp", bufs=out_bufs))
        dp = ctx.enter_context(tc.tile_pool(name="scr", bufs=2))
        pp = ctx.enter_context(tc.tile_pool(name="ps", bufs=ps_bufs, space="PSUM"))

        g8_t = cpool.tile([128, QCOLS], F32)
        nc.sync.dma_start(g8_t[:], g8[:, :])
        iva_t = cpool.tile([128, NA * QCOLS], F32)
        nc.sync.dma_start(iva_t[:], iva[:, :])
        id_t = cpool.tile([NCH, NCH], F32)
        nc.sync.dma_start(id_t[:], idn[:, :])
        id32_t = None
        if base_alt:
            # second identity copy at partition base 32 (PE requires lhsT and
            # rhs to share base partition). NOTE: dead on TRN2 -- APs starting
            # at partition 32 may span at most 32 partitions, so 85-row slabs
            # can only sit at base 0.
            id32_t = cpool.tile([32 + NCH, NCH], F32)
            nc.sync.dma_start(id32_t[32 : 32 + NCH, :], idn[:, :])

        bounds = [GG * c // sig_chunks for c in range(sig_chunks + 1)]
        in_eng = getattr(nc, in_engine) if in_engine != "alt" else nc.scalar
        for b in range(nb):
            # Stage this batch's channels in SBUF with full partition width
            # (16 SBUF ports want 128 partitions) and sigmoid them in place.
            if wide_in:
                x0 = inp.tile([128, GG], F32, tag="x0")
                x1 = inp.tile([127, GG], F32, tag="x1")
                for lo, hi in zip(bounds, bounds[1:]):
                    in_eng.dma_start(x0[:, lo:hi], x[b][0:128, lo:hi])
                    in_eng.dma_start(x1[:, lo:hi], x[b][128:255, lo:hi])
                for lo, hi in zip(bounds, bounds[1:]):
                    nc.scalar.activation(x0[:, lo:hi], x0[:, lo:hi], Act.Sigmoid)
                    nc.scalar.activation(x1[:, lo:hi], x1[:, lo:hi], Act.Sigmoid)
                # anchor a rows [85a, 85a+85) -> (tile, row_off, ch_off, cnt)
                srcs = {
                    0: [(x0, 0, 0, NCH)],
                    1: [(x0, 85, 0, 43), (x1, 0, 43, 42)],
                    2: [(x1, 42, 0, NCH)],
                }
            for a in range(NA):
                if wide_in:
                    asrc = srcs[a]
                    a_id = id_t
                else:
                    off = 32 if (base_alt and (b * NA + a) % 2 == 1) else 0
                    xin_f = inp.tile([32 + NCH, GG], F32, tag="xin")
                    xin = xin_f[off : off + NCH]
                    if in_engine == "alt":
                        in_eng = nc.scalar if (b * NA + a) % 2 == 0 else nc.gpsimd
                    for lo, hi in zip(bounds, bounds[1:]):
                        in_eng.dma_start(
                            xin[:, lo:hi], x[b][a * NCH : (a + 1) * NCH, lo:hi]
                        )
                    s_f = sp.tile([32 + NCH, GG], F32, tag="s")
                    s = s_f[off : off + NCH]
                    for lo, hi in zip(bounds, bounds[1:]):
                        nc.scalar.activation(s[:, lo:hi], xin[:, lo:hi], Act.Sigmoid)
                    asrc = [(s, 0, 0, NCH)]
                    a_id = id32_t[32 : 32 + NCH] if off == 32 else id_t

                o = op.tile([128, OUT_COLS], F32, tag="o")
                for j in range(NJ):
                    ps = pp.tile([128, MAIN_COLS], F32, tag="ps")
                    for kk in range(KK):
                        sel = slice(j * 768 + kk, (j + 1) * 768, KK)
                        for st, ro, co, cnt in asrc:
                            nc.tensor.transpose(
                                ps[:, kk * NCH + co : kk * NCH + co + cnt],
                                st[ro : ro + cnt, sel],
                                a_id[0:cnt, 0:cnt],
                            )
                    dst = o[:, j * MAIN_COLS : (j + 1) * MAIN_COLS]
                    if copy_split and j % 2 == 1:
                        nc.scalar.copy(dst, ps[:])
                    else:
                        nc.vector.tensor_copy(dst, ps[:])
                pst = pp.tile([128, MAIN_COLS], F32, tag="ps")
                for kk in range(TK):
                    sel = slice(MAIN_PX + kk, GG, TK)
                    for st, ro, co, cnt in asrc:
                        nc.tensor.transpose(
                            pst[0:TI, kk * NCH + co : kk * NCH + co + cnt],
                            st[ro : ro + cnt, sel],
                            a_id[0:cnt, 0:cnt],
                        )
                nc.vector.tensor_copy(
                    o[0:TI, NJ * MAIN_COLS : OUT_COLS], pst[0:TI, 0:TAIL_COLS]
                )

                # Box fix-ups in the transposed layout.
                # cols 0:2 -> (sigmoid * 8) + grid8 ; cols 2:4 ->
                # a*exp(w) = s*a/(1-s): d=(s-1)/a, r=1/d, out=(-s)*r.
                d = dp.tile([128, QCOLS], F32, tag="d")
                mv = o[:, 0 : NJ * MAIN_COLS].rearrange(
                    "p (j kk c) -> p j kk c", j=NJ, kk=KK, c=NCH
                )
                c01 = mv[:, :, :, 0:2]
                c23 = mv[:, :, :, 2:4]
                gm = g8_t[:, 0:84].rearrange(
                    "p (j kk c) -> p j kk c", j=NJ, kk=KK, c=2
                )
                im = iva_t[:, a * QCOLS : a * QCOLS + 84].rearrange(
                    "p (j kk c) -> p j kk c", j=NJ, kk=KK, c=2
                )
                dm = d[:, 0:84].rearrange("p (j kk c) -> p j kk c", j=NJ, kk=KK, c=2)
                nc.vector.scalar_tensor_tensor(c01, c01, STRIDE, gm, Alu.mult, Alu.add)
                nc.vector.scalar_tensor_tensor(
                    dm, c23, 1.0, im, Alu.subtract, Alu.mult
                )
                nc.vector.reciprocal(d[:, 0:84], d[:, 0:84])
                nc.vector.scalar_tensor_tensor(c23, c23, -1.0, dm, Alu.mult, Alu.mult)

                tv = o[0:TI, NJ * MAIN_COLS : OUT_COLS].rearrange(
                    "p (kk c) -> p kk c", kk=TK, c=NCH
                )
                t01 = tv[:, :, 0:2]
                t23 = tv[:, :, 2:4]
                gt = g8_t[0:TI, 84:QCOLS].rearrange("p (kk c) -> p kk c", kk=TK, c=2)
                it = iva_t[0:TI, a * QCOLS + 84 : (a + 1) * QCOLS].rearrange(
                    "p (kk c) -> p kk c", kk=TK, c=2
                )
                dt = d[0:TI, 84:QCOLS].rearrange("p (kk c) -> p kk c", kk=TK, c=2)
                nc.vector.scalar_tensor_tensor(t01, t01, STRIDE, gt, Alu.mult, Alu.add)
                nc.vector.scalar_tensor_tensor(
                    dt, t23, 1.0, it, Alu.subtract, Alu.mult
                )
                nc.vector.reciprocal(d[0:TI, 84:QCOLS], d[0:TI, 84:QCOLS])
                nc.vector.scalar_tensor_tensor(t23, t23, -1.0, dt, Alu.mult, Alu.mult)

                om = out[b, a][0:MAIN_PX].rearrange(
                    "(j i kk) c -> i j kk c", j=NJ, i=KI, kk=KK
                )
                nc.sync.dma_start(om, o[:, 0 : NJ * MAIN_COLS])
                ot = out[b, a][MAIN_PX:GG].rearrange("(i kk) c -> i kk c", i=TI, kk=TK)
                nc.sync.dma_start(ot, o[0:TI, NJ * MAIN_COLS : OUT_COLS])

    nc.compile()
    return nc


def _consts(anchors: np.ndarray):
    i128 = np.arange(128)
    grid8 = np.zeros((128, QCOLS), np.float32)
    for j in range(NJ):
        for kk in range(KK):
            p = j * KI * KK + i128 * KK + kk
            grid8[:, j * 12 + kk * 2 + 0] = STRIDE * (p % G)
            grid8[:, j * 12 + kk * 2 + 1] = STRIDE * (p // G)
    for kk in range(TK):
        p = MAIN_PX + i128[:TI] * TK + kk
        grid8[:TI, 84 + kk * 2 + 0] = STRIDE * (p % G)
        grid8[:TI, 84 + kk * 2 + 1] = STRIDE * (p // G)

    inva = np.zeros((128, NA * QCOLS), np.float32)
    for a in range(NA):
        for q in range(QCOLS):
            inva[:, a * QCOLS + q] = 1.0 / float(anchors[a][q % 2])

    ident = np.eye(NCH, dtype=np.float32)
    return grid8, inva, ident


_NC_CACHE: dict[int, object] = {}

LAST_RESULTS = None


def kernel(x: np.ndarray, anchors: np.ndarray) -> np.ndarray:
    global LAST_RESULTS
    x = np.ascontiguousarray(x, dtype=np.float32)
    anchors = np.asarray(anchors, dtype=np.float32)
    B = x.shape[0]
    nb = B // N_CORES
    assert nb * N_CORES == B

    if nb not in _NC_CACHE:
        _NC_CACHE[nb] = _build(nb)
    nc = _NC_CACHE[nb]

    grid8, inva, ident = _consts(anchors)
    xr = x.reshape(B, NA * NCH, GG)
    in_maps = [
        {
            "x": xr[c * nb : (c + 1) * nb],
            "grid8": grid8,
            "inva": inva,
            "ident": ident,
        }
        for c in range(N_CORES)
    ]
    res = run_bass_kernel_spmd(nc, in_maps, list(range(N_CORES)))
    LAST_RESULTS = res
    outs = [
        np.asarray(res.results[c]["out"]).reshape(nb, NA * GG, NCH)
        for c in range(N_CORES)
    ]
    return np.concatenate(outs, axis=0)


# revision 25
# speedup vs baseline: 1.0001x; 1.0001x over previous
"""Trainium2 Bass kernel for the YOLO-style DetectionLayer.

Reference computation (per batch b, anchor a, grid cell (gy, gx)):
    pred = x[b].reshape(3, 85, 76, 76)  channels-first per anchor
    bx = (sigmoid(tx) + gx) * stride        stride = 608/76 = 8
    by = (sigmoid(ty) + gy) * stride
    bw = exp(tw) * anchor_w                 (stride cancels)
    bh = exp(th) * anchor_h
    conf/cls = sigmoid(...)
    out[b, a*5776 + gy*76 + gx, :] = [bx, by, bw, bh, conf, cls0..79]

Strategy (pure data-parallel over batch, 8 cores x 4 images):
  * Per (b, a) slab: DMA [85 ch, 5776 px] -> SBUF (channels on partitions).
  * One ACT pass: sigmoid over all 85 rows (single table set for the whole
    kernel -- exp is derived on DVE as s/(1-s) to avoid the ~2.7us ACT
    table switch between the sigmoid and exp sets).
  * TensorE transpose-mode matmuls flip [85, 128px] -> PSUM [128px, 85ch].
    Pixels are interleaved stride-6 so each SBUF output partition holds 6
    consecutive output rows = 2040 contiguous bytes in DRAM per partition
    (ideal DMA burst size).
  * Box fix-ups run in the transposed layout where box channels are a few
    free-dim columns across all 128 partitions (3-4 DVE ops per slab).
  * One big store DMA per slab, fully contiguous destination.
"""

from contextlib import ExitStack

import numpy as np

import concourse.bacc as bacc
import concourse.mybir as mybir
import concourse.tile as tile
from concourse.bass_utils import run_bass_kernel_spmd

F32 = mybir.dt.float32
Alu = mybir.AluOpType
Act = mybir.ActivationFunctionType

N_CORES = 8
NA = 3  # anchors
NCH = 85  # 5 + 80 classes
G = 76
GG = G * G  # 5776
STRIDE = 8.0

# pixel chunking for the transpose: 7 chunks of 128 partitions x 6 px
# (stride-6 interleave), tail chunk of 100 partitions x 4 px.
NJ, KI, KK = 7, 128, 6  # main: 7 * 768 px
TI, TK = 100, 4  # tail: 400 px
MAIN_PX = NJ * KI * KK  # 5376
MAIN_COLS = KK * NCH  # 510
TAIL_COLS = TK * NCH  # 340
OUT_COLS = NJ * MAIN_COLS + TAIL_COLS  # 3910

# grid8 / inva column layout: main j<7: q = j*12 + kk*2 + c ; tail: 84 + kk*2 + c
QCOLS = NJ * KK * 2 + TK * 2  # 92


def _build(
    nb: int,
    inp_bufs: int = 2,
    sig_bufs: int = 2,
    out_bufs: int = 3,
    ps_bufs: int = 4,
    copy_split: bool = False,
    sig_chunks: int = 3,
    in_engine: str = "gpsimd",
    wide_in: bool = False,
    base_alt: bool = False,
):
    nc = bacc.Bacc(
        "TRN2", target_bir_lowering=False, debug=False, enable_asserts=False
    )
    x = nc.dram_tensor("x", [nb, NA * NCH, GG], F32, kind="ExternalInput")
    g8 = nc.dram_tensor("grid8", [128, QCOLS], F32, kind="ExternalInput")
    iva = nc.dram_tensor("inva", [128, NA * QCOLS], F32, kind="ExternalInput")
    idn = nc.dram_tensor("ident", [NCH, NCH], F32, kind="ExternalInput")
    out = nc.dram_tensor("out", [nb, NA, GG, NCH], F32, kind="ExternalOutput")

    with tile.TileContext(nc) as tc, ExitStack() as ctx:
        cpool = ctx.enter_context(tc.tile_pool(name="consts", bufs=1))
        inp = ctx.enter_context(tc.tile_pool(name="inp", bufs=inp_bufs))
        sp = ctx.enter_context(tc.tile_pool(name="sig", bufs=sig_bufs))
        op = ctx.enter_context(tc.tile_pool(name="outp", bufs=out_bufs))
        dp = ctx.enter_context(tc.tile_pool(name="scr", bufs=2))
        pp = ctx.enter_context(tc.tile_pool(name="ps", bufs=ps_bufs, space="PSUM"))

        g8_t = cpool.tile([128, QCOLS], F32)
        nc.sync.dma_start(g8_t[:], g8[:, :])
        iva_t = cpool.tile([128, NA * QCOLS], F32)
        nc.sync.dma_start(iva_t[:], iva[:, :])
        id_t = cpool.tile([NCH, NCH], F32)
        nc.sync.dma_start(id_t[:], idn[:, :])
        id32_t = None
        if base_alt:
            # second identity copy at partition base 32 (PE requires lhsT and
            # rhs to share base partition). NOTE: dead on TRN2 -- APs starting
            # at partition 32 may span at most 32 partitions, so 85-row slabs
            # can only sit at base 0.
            id32_t = cpool.tile([32 + NCH, NCH], F32)
            nc.sync.dma_start(id32_t[32 : 32 + NCH, :], idn[:, :])

        bounds = [GG * c // sig_chunks for c in range(sig_chunks + 1)]
        in_eng = getattr(nc, in_engine) if in_engine != "alt" else nc.scalar
        for b in range(nb):
            # Stage this batch's channels in SBUF with full partition width
            # (16 SBUF ports want 128 partitions) and sigmoid them in place.
            if wide_in:
                x0 = inp.tile([128, GG], F32, tag="x0")
                x1 = inp.tile([127, GG], F32, tag="x1")
                for lo, hi in zip(bounds, bounds[1:]):
                    in_eng.dma_start(x0[:, lo:hi], x[b][0:128, lo:hi])
                    in_eng.dma_start(x1[:, lo:hi], x[b][128:255, lo:hi])
                for lo, hi in zip(bounds, bounds[1:]):
                    nc.scalar.activation(x0[:, lo:hi], x0[:, lo:hi], Act.Sigmoid)
                    nc.scalar.activation(x1[:, lo:hi], x1[:, lo:hi], Act.Sigmoid)
                # anchor a rows [85a, 85a+85) -> (tile, row_off, ch_off, cnt)
                srcs = {
                    0: [(x0, 0, 0, NCH)],
                    1: [(x0, 85, 0, 43), (x1, 0, 43, 42)],
                    2: [(x1, 42, 0, NCH)],
                }
            for a in range(NA):
                if wide_in:
                    asrc = srcs[a]
                    a_id = id_t
                else:
                    off = 32 if (base_alt and (b * NA + a) % 2 == 1) else 0
                    xin_f = inp.tile([32 + NCH, GG], F32, tag="xin")
                    xin = xin_f[off : off + NCH]
                    if in_engine == "alt":
                        in_eng = nc.scalar if (b * NA + a) % 2 == 0 else nc.gpsimd
                    for lo, hi in zip(bounds, bounds[1:]):
                        in_eng.dma_start(
                            xin[:, lo:hi], x[b][a * NCH : (a + 1) * NCH, lo:hi]
                        )
                    s_f = sp.tile([32 + NCH, GG], F32, tag="s")
                    s = s_f[off : off + NCH]
                    for lo, hi in zip(bounds, bounds[1:]):
                        nc.scalar.activation(s[:, lo:hi], xin[:, lo:hi], Act.Sigmoid)
                    asrc = [(s, 0, 0, NCH)]
                    a_id = id32_t[32 : 32 + NCH] if off == 32 else id_t

                o = op.tile([128, OUT_COLS], F32, tag="o")
                for j in range(NJ):
                    ps = pp.tile([128, MAIN_COLS], F32, tag="ps")
                    for kk in range(KK):
                        sel = slice(j * 768 + kk, (j + 1) * 768, KK)
                        for st, ro, co, cnt in asrc:
                            nc.tensor.transpose(
                                ps[:, kk * NCH + co : kk * NCH + co + cnt],
                                st[ro : ro + cnt, sel],
                                a_id[0:cnt, 0:cnt],
                            )
                    dst = o[:, j * MAIN_COLS : (j + 1) * MAIN_COLS]
                    if copy_split and j % 2 == 1:
                        nc.scalar.copy(dst, ps[:])
                    else:
                        nc.vector.tensor_copy(dst, ps[:])
                pst = pp.tile([128, MAIN_COLS], F32, tag="ps")
                for kk in range(TK):
                    sel = slice(MAIN_PX + kk, GG, TK)
                    for st, ro, co, cnt in asrc:
                        nc.tensor.transpose(
                            pst[0:TI, kk * NCH + co : kk * NCH + co + cnt],
                            st[ro : ro + cnt, sel],
                            a_id[0:cnt, 0:cnt],
                        )
                nc.vector.tensor_copy(
                    o[0:TI, NJ * MAIN_COLS : OUT_COLS], pst[0:TI, 0:TAIL_COLS]
                )

                # Box fix-ups in the transposed layout.
                # cols 0:2 -> (sigmoid * 8) + grid8 ; cols 2:4 ->
                # a*exp(w) = s*a/(1-s): d=(s-1)/a, r=1/d, out=(-s)*r.
                d = dp.tile([128, QCOLS], F32, tag="d")
                mv = o[:, 0 : NJ * MAIN_COLS].rearrange(
                    "p (j kk c) -> p j kk c", j=NJ, kk=KK, c=NCH
                )
                c01 = mv[:, :, :, 0:2]
                c23 = mv[:, :, :, 2:4]
                gm = g8_t[:, 0:84].rearrange(
                    "p (j kk c) -> p j kk c", j=NJ, kk=KK, c=2
                )
                im = iva_t[:, a * QCOLS : a * QCOLS + 84].rearrange(
                    "p (j kk c) -> p j kk c", j=NJ, kk=KK, c=2
                )
                dm = d[:, 0:84].rearrange("p (j kk c) -> p j kk c", j=NJ, kk=KK, c=2)
                nc.vector.scalar_tensor_tensor(c01, c01, STRIDE, gm, Alu.mult, Alu.add)
                nc.vector.scalar_tensor_tensor(
                    dm, c23, 1.0, im, Alu.subtract, Alu.mult
                )
                nc.vector.reciprocal(d[:, 0:84], d[:, 0:84])
                nc.vector.scalar_tensor_tensor(c23, c23, -1.0, dm, Alu.mult, Alu.mult)

                tv = o[0:TI, NJ * MAIN_COLS : OUT_COLS].rearrange(
                    "p (kk c) -> p kk c", kk=TK, c=NCH
                )
                t01 = tv[:, :, 0:2]
                t23 = tv[:, :, 2:4]
                gt = g8_t[0:TI, 84:QCOLS].rearrange("p (kk c) -> p kk c", kk=TK, c=2)
                it = iva_t[0:TI, a * QCOLS + 84 : (a + 1) * QCOLS].rearrange(
                    "p (kk c) -> p kk c", kk=TK, c=2
                )
                dt = d[0:TI, 84:QCOLS].rearrange("p (kk c) -> p kk c", kk=TK, c=2)
                nc.vector.scalar_tensor_tensor(t01, t01, STRIDE, gt, Alu.mult, Alu.add)
                nc.vector.scalar_tensor_tensor(
                    dt, t23, 1.0, it, Alu.subtract, Alu.mult
                )
                nc.vector.reciprocal(d[0:TI, 84:QCOLS], d[0:TI, 84:QCOLS])
                nc.vector.scalar_tensor_tensor(t23, t23, -1.0, dt, Alu.mult, Alu.mult)

                om = out[b, a][0:MAIN_PX].rearrange(
                    "(j i kk) c -> i j kk c", j=NJ, i=KI, kk=KK
                )
                nc.sync.dma_start(om, o[:, 0 : NJ * MAIN_COLS])
                ot = out[b, a][MAIN_PX:GG].rearrange("(i kk) c -> i kk c", i=TI, kk=TK)
                nc.sync.dma_start(ot, o[0:TI, NJ * MAIN_COLS : OUT_COLS])

    nc.compile()
    return nc


def _consts(anchors: np.ndarray):
    i128 = np.arange(128)
    grid8 = np.zeros((128, QCOLS), np.float32)
    for j in range(NJ):
        for kk in range(KK):
            p = j * KI * KK + i128 * KK + kk
            grid8[:, j * 12 + kk * 2 + 0] = STRIDE * (p % G)
            grid8[:, j * 12 + kk * 2 + 1] = STRIDE * (p // G)
    for kk in range(TK):
        p = MAIN_PX + i128[:TI] * TK + kk
        grid8[:TI, 84 + kk * 2 + 0] = STRIDE * (p % G)
        grid8[:TI, 84 + kk * 2 + 1] = STRIDE * (p // G)

    inva = np.zeros((128, NA * QCOLS), np.float32)
    for a in range(NA):
        for q in range(QCOLS):
            inva[:, a * QCOLS + q] = 1.0 / float(anchors[a][q % 2])

    ident = np.eye(NCH, dtype=np.float32)
    return grid8, inva, ident


_NC_CACHE: dict[int, object] = {}

LAST_RESULTS = None


def kernel(x: np.ndarray, anchors: np.ndarray) -> np.ndarray:
    global LAST_RESULTS
    x = np.ascontiguousarray(x, dtype=np.float32)
    anchors = np.asarray(anchors, dtype=np.float32)
    B = x.shape[0]
    nb = B // N_CORES
    assert nb * N_CORES == B

    if nb not in _NC_CACHE:
        _NC_CACHE[nb] = _build(nb)
    nc = _NC_CACHE[nb]

    grid8, inva, ident = _consts(anchors)
    xr = x.reshape(B, NA * NCH, GG)
    in_maps = [
        {
            "x": xr[c * nb : (c + 1) * nb],
            "grid8": grid8,
            "inva": inva,
            "ident": ident,
        }
        for c in range(N_CORES)
    ]
    res = run_bass_kernel_spmd(nc, in_maps, list(range(N_CORES)))
    LAST_RESULTS = res
    outs = [
        np.asarray(res.results[c]["out"]).reshape(nb, NA * GG, NCH)
        for c in range(N_CORES)
    ]
    return np.concatenate(outs, axis=0)


# revision 31
# speedup vs baseline: 1.0016x; 1.0015x over previous
"""Trainium2 Bass kernel for the YOLO-style DetectionLayer.

Reference computation (per batch b, anchor a, grid cell (gy, gx)):
    pred = x[b].reshape(3, 85, 76, 76)  channels-first per anchor
    bx = (sigmoid(tx) + gx) * stride        stride = 608/76 = 8
    by = (sigmoid(ty) + gy) * stride
    bw = exp(tw) * anchor_w                 (stride cancels)
    bh = exp(th) * anchor_h
    conf/cls = sigmoid(...)
    out[b, a*5776 + gy*76 + gx, :] = [bx, by, bw, bh, conf, cls0..79]

Strategy (pure data-parallel over batch, 8 cores x 4 images):
  * Per (b, a) slab: DMA [85 ch, 5776 px] -> SBUF (channels on partitions).
  * One ACT pass: sigmoid over all 85 rows (single table set for the whole
    kernel -- exp is derived on DVE as s/(1-s) to avoid the ~2.7us ACT
    table switch between the sigmoid and exp sets).
  * TensorE transpose-mode matmuls flip [85, 128px] -> PSUM [128px, 85ch].
    Pixels are interleaved stride-6 so each SBUF output partition holds 6
    consecutive output rows = 2040 contiguous bytes in DRAM per partition
    (ideal DMA burst size).
  * Box fix-ups run in the transposed layout where box channels are a few
    free-dim columns across all 128 partitions (3-4 DVE ops per slab).
  * One big store DMA per slab, fully contiguous destination.
"""

from contextlib import ExitStack

import numpy as np

import concourse.bacc as bacc
import concourse.mybir as mybir
import concourse.tile as tile
from concourse.bass_utils import run_bass_kernel_spmd

F32 = mybir.dt.float32
Alu = mybir.AluOpType
Act = mybir.ActivationFunctionType

N_CORES = 8
NA = 3  # anchors
NCH = 85  # 5 + 80 classes
G = 76
GG = G * G  # 5776
STRIDE = 8.0

# pixel chunking for the transpose: 7 chunks of 128 partitions x 6 px
# (stride-6 interleave), tail chunk of 100 partitions x 4 px.
NJ, KI, KK = 7, 128, 6  # main: 7 * 768 px
TI, TK = 100, 4  # tail: 400 px
MAIN_PX = NJ * KI * KK  # 5376
MAIN_COLS = KK * NCH  # 510
TAIL_COLS = TK * NCH  # 340
OUT_COLS = NJ * MAIN_COLS + TAIL_COLS  # 3910

# grid8 / inva column layout: main j<7: q = j*12 + kk*2 + c ; tail: 84 + kk*2 + c
QCOLS = NJ * KK * 2 + TK * 2  # 92


def _build(
    nb: int,
    inp_bufs: int = 2,
    sig_bufs: int = 2,
    out_bufs: int = 3,
    ps_bufs: int = 4,
    copy_split: bool = False,
    sig_chunks: int = 3,
    in_engine: str = "gpsimd",
    wide_in: bool = False,
    base_alt: bool = False,
):
    nc = bacc.Bacc(
        "TRN2", target_bir_lowering=False, debug=False, enable_asserts=False
    )
    x = nc.dram_tensor("x", [nb, NA * NCH, GG], F32, kind="ExternalInput")
    # all constants packed in one tensor so the single const DMA has
    # >=512B per-partition runs (small separate consts pay the sub-512B
    # 2x descriptor penalty) and mostly fits in the boot shadow.
    # cols 0:92 grid8 | 92:368 inva | 368:453 ident (rows 0:85)
    CP = QCOLS + NA * QCOLS + NCH  # 453
    cpk = nc.dram_tensor("cpack", [128, CP], F32, kind="ExternalInput")
    out = nc.dram_tensor("out", [nb, NA, GG, NCH], F32, kind="ExternalOutput")

    with tile.TileContext(nc) as tc, ExitStack() as ctx:
        cpool = ctx.enter_context(tc.tile_pool(name="consts", bufs=1))
        inp = ctx.enter_context(tc.tile_pool(name="inp", bufs=inp_bufs))
        sp = ctx.enter_context(tc.tile_pool(name="sig", bufs=sig_bufs))
        op = ctx.enter_context(tc.tile_pool(name="outp", bufs=out_bufs))
        dp = ctx.enter_context(tc.tile_pool(name="scr", bufs=2))
        pp = ctx.enter_context(tc.tile_pool(name="ps", bufs=ps_bufs, space="PSUM"))

        assert not base_alt, "dead on TRN2: base-32 APs span at most 32 partitions"
        cp_t = cpool.tile([128, CP], F32)
        nc.sync.dma_start(cp_t[:], cpk[:, :])
        g8_t = cp_t[:, 0:QCOLS]
        iva_t = cp_t[:, QCOLS : QCOLS + NA * QCOLS]
        id_t = cp_t[0:NCH, QCOLS + NA * QCOLS : CP]

        bounds = [GG * c // sig_chunks for c in range(sig_chunks + 1)]
        in_eng = getattr(nc, in_engine) if in_engine != "alt" else nc.scalar
        for b in range(nb):
            # Stage this batch's channels in SBUF with full partition width
            # (16 SBUF ports want 128 partitions) and sigmoid them in place.
            if wide_in:
                x0 = inp.tile([128, GG], F32, tag="x0")
                x1 = inp.tile([127, GG], F32, tag="x1")
                for lo, hi in zip(bounds, bounds[1:]):
                    in_eng.dma_start(x0[:, lo:hi], x[b][0:128, lo:hi])
                    in_eng.dma_start(x1[:, lo:hi], x[b][128:255, lo:hi])
                for lo, hi in zip(bounds, bounds[1:]):
                    nc.scalar.activation(x0[:, lo:hi], x0[:, lo:hi], Act.Sigmoid)
                    nc.scalar.activation(x1[:, lo:hi], x1[:, lo:hi], Act.Sigmoid)
                # anchor a rows [85a, 85a+85) -> (tile, row_off, ch_off, cnt)
                srcs = {
                    0: [(x0, 0, 0, NCH)],
                    1: [(x0, 85, 0, 43), (x1, 0, 43, 42)],
                    2: [(x1, 42, 0, NCH)],
                }
            for a in range(NA):
                if wide_in:
                    asrc = srcs[a]
                    a_id = id_t
                else:
                    off = 32 if (base_alt and (b * NA + a) % 2 == 1) else 0
                    xin_f = inp.tile([32 + NCH, GG], F32, tag="xin")
                    xin = xin_f[off : off + NCH]
                    if in_engine == "alt":
                        in_eng = nc.scalar if (b * NA + a) % 2 == 0 else nc.gpsimd
                    for lo, hi in zip(bounds, bounds[1:]):
                        in_eng.dma_start(
                            xin[:, lo:hi], x[b][a * NCH : (a + 1) * NCH, lo:hi]
                        )
                    s_f = sp.tile([32 + NCH, GG], F32, tag="s")
                    s = s_f[off : off + NCH]
                    for lo, hi in zip(bounds, bounds[1:]):
                        nc.scalar.activation(s[:, lo:hi], xin[:, lo:hi], Act.Sigmoid)
                    asrc = [(s, 0, 0, NCH)]
                    a_id = id_t

                o = op.tile([128, OUT_COLS], F32, tag="o")
                for j in range(NJ):
                    ps = pp.tile([128, MAIN_COLS], F32, tag="ps")
                    for kk in range(KK):
                        sel = slice(j * 768 + kk, (j + 1) * 768, KK)
                        for st, ro, co, cnt in asrc:
                            nc.tensor.transpose(
                                ps[:, kk * NCH + co : kk * NCH + co + cnt],
                                st[ro : ro + cnt, sel],
                                a_id[0:cnt, 0:cnt],
                            )
                    dst = o[:, j * MAIN_COLS : (j + 1) * MAIN_COLS]
                    if copy_split and j % 2 == 1:
                        nc.scalar.copy(dst, ps[:])
                    else:
                        nc.vector.tensor_copy(dst, ps[:])
                pst = pp.tile([128, MAIN_COLS], F32, tag="ps")
                for kk in range(TK):
                    sel = slice(MAIN_PX + kk, GG, TK)
                    for st, ro, co, cnt in asrc:
                        nc.tensor.transpose(
                            pst[0:TI, kk * NCH + co : kk * NCH + co + cnt],
                            st[ro : ro + cnt, sel],
                            a_id[0:cnt, 0:cnt],
                        )
                nc.vector.tensor_copy(
                    o[0:TI, NJ * MAIN_COLS : OUT_COLS], pst[0:TI, 0:TAIL_COLS]
                )

                # Box fix-ups in the transposed layout.
                # cols 0:2 -> (sigmoid * 8) + grid8 ; cols 2:4 ->
                # a*exp(w) = s*a/(1-s): d=(s-1)/a, r=1/d, out=(-s)*r.
                d = dp.tile([128, QCOLS], F32, tag="d")
                mv = o[:, 0 : NJ * MAIN_COLS].rearrange(
                    "p (j kk c) -> p j kk c", j=NJ, kk=KK, c=NCH
                )
                c01 = mv[:, :, :, 0:2]
                c23 = mv[:, :, :, 2:4]
                gm = g8_t[:, 0:84].rearrange(
                    "p (j kk c) -> p j kk c", j=NJ, kk=KK, c=2
                )
                im = iva_t[:, a * QCOLS : a * QCOLS + 84].rearrange(
                    "p (j kk c) -> p j kk c", j=NJ, kk=KK, c=2
                )
                dm = d[:, 0:84].rearrange("p (j kk c) -> p j kk c", j=NJ, kk=KK, c=2)
                nc.vector.scalar_tensor_tensor(c01, c01, STRIDE, gm, Alu.mult, Alu.add)
                nc.vector.scalar_tensor_tensor(
                    dm, c23, 1.0, im, Alu.subtract, Alu.mult
                )
                nc.vector.reciprocal(d[:, 0:84], d[:, 0:84])
                nc.vector.scalar_tensor_tensor(c23, c23, -1.0, dm, Alu.mult, Alu.mult)

                tv = o[0:TI, NJ * MAIN_COLS : OUT_COLS].rearrange(
                    "p (kk c) -> p kk c", kk=TK, c=NCH
                )
                t01 = tv[:, :, 0:2]
                t23 = tv[:, :, 2:4]
                gt = g8_t[0:TI, 84:QCOLS].rearrange("p (kk c) -> p kk c", kk=TK, c=2)
                it = iva_t[0:TI, a * QCOLS + 84 : (a + 1) * QCOLS].rearrange(
                    "p (kk c) -> p kk c", kk=TK, c=2
                )
                dt = d[0:TI, 84:QCOLS].rearrange("p (kk c) -> p kk c", kk=TK, c=2)
                nc.vector.scalar_tensor_tensor(t01, t01, STRIDE, gt, Alu.mult, Alu.add)
                nc.vector.scalar_tensor_tensor(
                    dt, t23, 1.0, it, Alu.subtract, Alu.mult
                )
                nc.vector.reciprocal(d[0:TI, 84:QCOLS], d[0:TI, 84:QCOLS])
                nc.vector.scalar_tensor_tensor(t23, t23, -1.0, dt, Alu.mult, Alu.mult)

                om = out[b, a][0:MAIN_PX].rearrange(
                    "(j i kk) c -> i j kk c", j=NJ, i=KI, kk=KK
                )
                nc.sync.dma_start(om, o[:, 0 : NJ * MAIN_COLS])
                ot = out[b, a][MAIN_PX:GG].rearrange("(i kk) c -> i kk c", i=TI, kk=TK)
                nc.sync.dma_start(ot, o[0:TI, NJ * MAIN_COLS : OUT_COLS])

    nc.compile()
    return nc


def _consts(anchors: np.ndarray):
    i128 = np.arange(128)
    grid8 = np.zeros((128, QCOLS), np.float32)
    for j in range(NJ):
        for kk in range(KK):
            p = j * KI * KK + i128 * KK + kk
            grid8[:, j * 12 + kk * 2 + 0] = STRIDE * (p % G)
            grid8[:, j * 12 + kk * 2 + 1] = STRIDE * (p // G)
    for kk in range(TK):
        p = MAIN_PX + i128[:TI] * TK + kk
        grid8[:TI, 84 + kk * 2 + 0] = STRIDE * (p % G)
        grid8[:TI, 84 + kk * 2 + 1] = STRIDE * (p // G)

    inva = np.zeros((128, NA * QCOLS), np.float32)
    for a in range(NA):
        for q in range(QCOLS):
            inva[:, a * QCOLS + q] = 1.0 / float(anchors[a][q % 2])

    ident = np.eye(NCH, dtype=np.float32)

    cpack = np.zeros((128, QCOLS + NA * QCOLS + NCH), np.float32)
    cpack[:, 0:QCOLS] = grid8
    cpack[:, QCOLS : QCOLS + NA * QCOLS] = inva
    cpack[0:NCH, QCOLS + NA * QCOLS :] = ident
    return cpack


_NC_CACHE: dict[int, object] = {}

LAST_RESULTS = None


def kernel(x: np.ndarray, anchors: np.ndarray) -> np.ndarray:
    global LAST_RESULTS
    x = np.ascontiguousarray(x, dtype=np.float32)
    anchors = np.asarray(anchors, dtype=np.float32)
    B = x.shape[0]
    nb = B // N_CORES
    assert nb * N_CORES == B

    if nb not in _NC_CACHE:
        _NC_CACHE[nb] = _build(nb)
    nc = _NC_CACHE[nb]

    cpack = _consts(anchors)
    xr = x.reshape(B, NA * NCH, GG)
    in_maps = [
        {"x": xr[c * nb : (c + 1) * nb], "cpack": cpack} for c in range(N_CORES)
    ]
    res = run_bass_kernel_spmd(nc, in_maps, list(range(N_CORES)))
    LAST_RESULTS = res
    outs = [
        np.asarray(res.results[c]["out"]).reshape(nb, NA * GG, NCH)
        for c in range(N_CORES)
    ]
    return np.concatenate(outs, axis=0)
